# revision 24
# baseline (speedup 1.0000x reference)
"""Fused cross-attention kernel for Trainium2, data-parallel over batch on 8 cores.

Per core (one batch element), using the low-rank structure of cross-attention
(L=77 << D=512):
  W2_h = Wq_h @ K_h^T  [512, 77] -> scores_h = X @ W2_h     (fp8 DoubleRow)
  W3_h = V_h @ Wo_h    [77, 512] -> out = sum_h attn_h^T.T @ W3_h  (fp16)
Main loop processes PAIRS of 128-row q-tiles through a 7-stage software
pipeline so the PE never waits on the softmax chain, and per-instruction
fixed costs (scalar ACTIVATE is (N+352)/1.2 ns) amortize over 2 tiles:
  S0 scores (PE)  S1 exp x2 (Scalar)  S2 reduce (Vector) + negcnt (GpSimd)
  S3 recip (Vector) + normalize (GpSimd)  S4 transposes (PE) + copy (Scalar)
  S5 out matmul (PE)  S6 out copies (Vector) + DMA
"""

import sys

sys.path.insert(0, "/opt/trn_rl_repo")

import numpy as np
import ml_dtypes

import concourse.bass as bass
import concourse.mybir as mybir
import concourse.tile as tile
from concourse import bacc
from concourse.bass_utils import run_bass_kernel_spmd
from concourse.masks import make_identity

N_CORES = 8
B, T, S_, D, L, H = 8, 64, 196, 512, 77, 4
DH = D // H  # 128
NQ = T * S_  # 12544
LN_EPS = 1e-6
SCALE = float(DH) ** -0.5
P = 128
NCH = D // P  # 4 chunks of the feature dim
W2S = 8.0  # fp8 pre-scale for W2 (folded back out via exp's scale)
GT = 4  # q-tiles per DMA group (2KB per partition line)

F32 = mybir.dt.float32
BF16 = mybir.dt.bfloat16
FP16 = mybir.dt.float16
FP8 = mybir.dt.float8e4
DR = mybir.MatmulPerfMode.DoubleRow

LAST_RESULTS = None  # BassKernelResults of the most recent run (for test harness)
_PROGRAM_CACHE = {}


def build_program(nq=NQ, l_eff=L):
    nc = bacc.Bacc("TRN2", target_bir_lowering=False, debug=False, num_devices=N_CORES)

    LE = l_eff
    xt = nc.dram_tensor("xt", [nq * D], FP8, kind="ExternalInput").ap()
    text = nc.dram_tensor("text", [P, D], F32, kind="ExternalInput").ap()
    negcnt = nc.dram_tensor("negcnt", [P, 1], F32, kind="ExternalInput").ap()
    wqt = nc.dram_tensor("wqt", [P, H, NCH, P], FP16, kind="ExternalInput").ap()
    wk = nc.dram_tensor("wk", [D, D], FP16, kind="ExternalInput").ap()
    wv = nc.dram_tensor("wv", [D, D], FP16, kind="ExternalInput").ap()
    wo = nc.dram_tensor("wo", [D, D], FP16, kind="ExternalInput").ap()
    out = nc.dram_tensor("out", [nq, D], FP16, kind="ExternalOutput").ap()

    ntiles = nq // P  # 98
    npairs = ntiles // 2  # 49
    ngroups = (ntiles + GT - 1) // GT

    def group_span(g):
        t0 = g * GT
        return t0, min(GT, ntiles - t0)

    with tile.TileContext(nc) as tc:
        with (
            tc.tile_pool(name="const", bufs=1) as const,
            tc.tile_pool(name="xtp", bufs=3) as xtp,
            tc.tile_pool(name="ep", bufs=4) as ep,
            tc.tile_pool(name="atp", bufs=3) as atp,
            tc.tile_pool(name="attp", bufs=3) as attp,
            tc.tile_pool(name="smalls", bufs=24) as smalls,
            tc.tile_pool(name="outp", bufs=3) as outp,
            tc.tile_pool(name="ps_sc", bufs=4, space="PSUM") as ps_sc,
            tc.tile_pool(name="ps_at", bufs=2, space="PSUM") as ps_at,
            tc.tile_pool(name="ps_out", bufs=2, space="PSUM") as ps_out,
        ):
            # ---- constants / prolog ----
            ident16 = const.tile([P, P], FP16)
            make_identity(nc, ident16)
            wk_sb = const.tile([P, NCH, D], FP16, tag="wk")
            wv_sb = const.tile([P, NCH, D], FP16, tag="wv")
            wo_sb = const.tile([P, NCH, D], FP16, tag="wo")
            text_sb = const.tile([P, D], F32, tag="text")
            nc.sync.dma_start(out=text_sb[:], in_=text)
            nc.sync.dma_start(
                out=wk_sb[:], in_=wk.rearrange("(c p) n -> p c n", p=P)
            )
            wqT_sb = const.tile([P, H, NCH, P], FP16, tag="wqT")
            nc.sync.dma_start(out=wqT_sb[:], in_=wqt)

            # X prefetch of the first two groups ahead of the wv/wo DMAs
            xt_tiles = {}

            def dma_group(g):
                t0, gt = group_span(g)
                xt_sb = xtp.tile([P, NCH, GT * P], FP8, tag="xt")
                qg = gt * P
                in_ap = bass.AP(
                    tensor=xt.tensor,
                    offset=t0 * P * D,
                    ap=[[NCH * qg, P], [qg, NCH], [1, qg]],
                )
                nc.sync.dma_start(out=xt_sb[:, :, :qg], in_=in_ap)
                xt_tiles[g] = xt_sb

            dma_group(0)
            dma_group(1)
            for w_sb, w_dram in ((wv_sb, wv), (wo_sb, wo)):
                nc.sync.dma_start(
                    out=w_sb[:], in_=w_dram.rearrange("(c p) n -> p c n", p=P)
                )

            negcnt_sb = const.tile([P, 1], F32, tag="negcnt")
            nc.sync.dma_start(out=negcnt_sb[:], in_=negcnt)

            # ---- LayerNorm of text (77 valid rows; pad rows are zeros) ----
            stats = smalls.tile([P, 6], F32, tag="bnstats")
            nc.vector.bn_stats(out=stats[:], in_=text_sb[:])
            mv = smalls.tile([P, 2], F32, tag="bnaggr")
            nc.vector.bn_aggr(out=mv[:], in_=stats[:])
            eps_sb = smalls.tile([P, 1], F32, tag="eps")
            nc.vector.memset(eps_sb[:], LN_EPS)
            std = smalls.tile([P, 1], F32, tag="std")
            nc.scalar.activation(
                std[:], mv[:, 1:2], mybir.ActivationFunctionType.Sqrt, bias=eps_sb[:]
            )
            rstd = smalls.tile([P, 1], F32, tag="rstd")
            nc.vector.reciprocal(rstd[:], std[:])
            tn_sb = const.tile([P, D], FP16, tag="tn")
            nc.vector.tensor_scalar(
                out=tn_sb[:],
                in0=text_sb[:],
                scalar1=mv[:, 0:1],
                scalar2=rstd[:],
                op0=mybir.AluOpType.subtract,
                op1=mybir.AluOpType.mult,
            )

            # ---- tn^T (D on partitions) ----
            tnT_sb = const.tile([P, NCH, P], FP16, tag="tnT")
            for c in range(NCH):
                ps_t = ps_at.tile([P, 2 * H * P], FP16, tag="ps_tr")
                nc.tensor.transpose(ps_t[:, :P], tn_sb[:, c * P : (c + 1) * P], ident16[:])
                nc.scalar.copy(tnT_sb[:, c, :], ps_t[:, :P])

            # ---- K^T (masked cols) and V^T, feature dim on partitions ----
            kt_sb = const.tile([P, H, LE], FP16, tag="kt")
            vt_sb = const.tile([P, H, LE], FP16, tag="vt")
            for w_sb_, dst in ((wk_sb, kt_sb), (wv_sb, vt_sb)):
                for dch in range(NCH):
                    ps_k = ps_sc.tile([P, H * L], F32, tag="ps_s")
                    for kc in range(NCH):
                        nc.tensor.matmul(
                            ps_k[:, :LE],
                            w_sb_[:, kc, dch * P : (dch + 1) * P],
                            tnT_sb[:, kc, :LE],
                            start=(kc == 0),
                            stop=(kc == NCH - 1),
                        )
                    nc.vector.tensor_copy(dst[:, dch, :], ps_k[:, :LE])

            # ---- W2_h = Wq_h @ K_h^T -> fp8 * W2S, layout [p, c, h*L] ----
            w2_sb = const.tile([P, NCH, H, LE], FP8, tag="w2")
            for h in range(H):
                for dch in range(NCH):
                    ps_w = ps_sc.tile([P, H * L], F32, tag="ps_s")
                    nc.tensor.matmul(
                        ps_w[:, :LE], wqT_sb[:, h, dch, :], kt_sb[:, h, :],
                        start=True, stop=True,
                    )
                    nc.scalar.activation(
                        w2_sb[:, dch, h, :], ps_w[:, :LE],
                        mybir.ActivationFunctionType.Copy, scale=W2S,
                    )

            # ---- W3: heads packed in pairs, head 2hp+s at partition offset
            # s*LE of tile hp (LE=64 keeps offsets in the legal {0,64} set),
            # so the out matmul contracts two heads in one 128-row matmul ----
            w3_sb = const.tile([P, 2, D], FP16, tag="w3")

            def build_w3():
                for hp in range(2):
                    ps_w = ps_out.tile([P, D], F32, tag="ps_o")
                    for s in range(2):
                        nc.tensor.matmul(
                            ps_w[s * LE : (s + 1) * LE],
                            vt_sb[:, 2 * hp + s, :],
                            wo_sb[:, 2 * hp + s, :],
                            start=True, stop=True,
                        )
                    nc.scalar.copy(w3_sb[:, hp, :], ps_w[:])

            # ---- main loop: pairs of q-tiles through a 7-stage pipeline ----
            E_t, attn_t, attnT_t, sum_t, adj_t, recip_t = {}, {}, {}, {}, {}, {}
            ps_s_t, ps_o_t = {}, {}

            for i in range(npairs + 7):
                if i == 4:
                    build_w3()
                # S0: scores on PE (fp8 DoubleRow), 2 tiles
                if i < npairs:
                    if i % 2 == 0:
                        gnext = i // 2 + 2
                        if gnext < ngroups:
                            dma_group(gnext)
                    ps_s = ps_sc.tile([P, 2 * H * LE], F32, tag="ps_s")
                    for jt, t in enumerate((2 * i, 2 * i + 1)):
                        g = t // GT
                        tq = slice((t - g * GT) * P, (t - g * GT + 1) * P)
                        for j in range(2):
                            nc.tensor.matmul(
                                ps_s[:, jt * H * LE : (jt + 1) * H * LE],
                                xt_tiles[g][:, 2 * j : 2 * j + 2, tq],
                                w2_sb[:, 2 * j : 2 * j + 2, :, :].rearrange(
                                    "p c h n -> p c (h n)"
                                ),
                                start=(j == 0),
                                stop=(j == 1),
                                perf_mode=DR,
                            )
                    ps_s_t[i] = ps_s

                # S1: exp x2 on Scalar -> E fp16 (both tiles of pair in one tile)
                p = i - 1
                if 0 <= p < npairs:
                    E_sb = ep.tile([P, 2, H, LE], FP16, tag="exp")
                    nc.scalar.activation(
                        E_sb[:],
                        ps_s_t[p][:].rearrange("p (j c n) -> p j c n", j=2, c=H),
                        mybir.ActivationFunctionType.Exp, scale=SCALE / W2S,
                    )
                    del ps_s_t[p]
                    E_t[p] = E_sb

                # S2: reduce on Vector; +negcnt on GpSimd
                p = i - 2
                if 0 <= p < npairs:
                    sumexp = smalls.tile([P, 2 * H], F32, tag="sumexp")
                    nc.vector.reduce_sum(
                        out=sumexp[:], in_=E_t[p][:], axis=mybir.AxisListType.X
                    )
                    sumadj = smalls.tile([P, 2 * H], F32, tag="sumadj")
                    nc.gpsimd.tensor_scalar_add(sumadj[:], sumexp[:], negcnt_sb[:])
                    adj_t[p] = sumadj

                # S3: recip on Vector; normalize on GpSimd -> attn fp16
                p = i - 3
                if 0 <= p < npairs:
                    recip = smalls.tile([P, 2 * H], F32, tag="recip")
                    nc.vector.reciprocal_approx_fast(recip[:], adj_t[p][:])
                    del adj_t[p]
                    attn_sb = atp.tile([P, 2, H, LE], FP16, tag="attn")
                    nc.gpsimd.tensor_mul(
                        attn_sb[:], E_t[p][:],
                        recip[:].rearrange("p (j c) -> p j c", j=2).to_broadcast(
                            [P, 2, H, LE]
                        ),
                    )
                    attn_t[p] = attn_sb
                    del E_t[p]

                # S4: transposes on PE (two heads per transpose: input
                # [128, 2*LE=128], output rows = packed head pair); copy on Scalar
                p = i - 4
                if 0 <= p < npairs:
                    ps_a = ps_at.tile([P, 2 * 2 * P], FP16, tag="ps_tr")
                    for j in range(2):
                        for hp in range(2):
                            sec = j * 2 + hp
                            nc.tensor.transpose(
                                ps_a[:, sec * P : (sec + 1) * P],
                                attn_t[p][:, j, 2 * hp : 2 * hp + 2, :].rearrange(
                                    "p a b -> p (a b)"
                                ),
                                ident16[:],
                            )
                    attnT_sb = attp.tile([P, 2, 2, P], FP16, tag="attnT")
                    nc.vector.tensor_copy(
                        attnT_sb[:].bitcast(mybir.dt.uint32),
                        ps_a[:].bitcast(mybir.dt.uint32),
                    )
                    attnT_t[p] = attnT_sb
                    del attn_t[p]

                # S5: out matmuls on PE (2 tiles, separate PSUM banks)
                p = i - 5
                if 0 <= p < npairs:
                    for j in range(2):
                        ps_o = ps_out.tile([P, D], F32, tag="ps_o")
                        for hp in range(2):
                            nc.tensor.matmul(
                                ps_o[:],
                                attnT_t[p][:, j, hp, :],
                                w3_sb[:, hp, :],
                                start=(hp == 0),
                                stop=(hp == 1),
                            )
                        ps_o_t[2 * p + j] = ps_o
                    del attnT_t[p]

                # S6: out copies on Vector, one pair-DMA out
                p = i - 6
                if 0 <= p < npairs:
                    out_sb = outp.tile([P, 2, D], FP16, tag="out")
                    nc.scalar.copy(out_sb[:, 0], ps_o_t[2 * p][:])
                    del ps_o_t[2 * p]
                    nc.vector.tensor_copy(
                        out_sb[:, 1, : D // 2], ps_o_t[2 * p + 1][:, : D // 2]
                    )
                    nc.scalar.copy(
                        out_sb[:, 1, D // 2 :], ps_o_t[2 * p + 1][:, D // 2 :]
                    )
                    del ps_o_t[2 * p + 1]
                    out_ap = bass.AP(
                        tensor=out.tensor,
                        offset=2 * p * P * D,
                        ap=[[D, P], [P * D, 2], [1, D]],
                    )
                    nc.sync.dma_start(out=out_ap, in_=out_sb[:])
                    g_done = (2 * p) // GT
                    if 2 * p + 1 == min(ntiles, (g_done + 1) * GT) - 1:
                        del xt_tiles[g_done]

    nc.compile()
    return nc


def _get_program(nq=NQ, l_eff=L):
    key = (nq, l_eff)
    if key not in _PROGRAM_CACHE:
        _PROGRAM_CACHE[key] = build_program(nq, l_eff)
    return _PROGRAM_CACHE[key]


def pack_xt_fp8(x):
    """[NQ, D] f32 -> flat fp8 in per-group [p, c, q] layout (2KB DMA lines)."""
    xtT = np.ascontiguousarray(x.T).astype(ml_dtypes.float8_e4m3fn)  # [D, NQ]
    ntiles = x.shape[0] // P
    flat = np.empty(x.size, ml_dtypes.float8_e4m3fn)
    t0 = 0
    while t0 < ntiles:
        gt = min(GT, ntiles - t0)
        q0, q1 = t0 * P, (t0 + gt) * P
        blk = xtT[:, q0:q1].reshape(NCH, P, gt * P).transpose(1, 0, 2)
        flat[q0 * D : q1 * D] = blk.ravel()
        t0 += gt
    return flat


def pick_l_eff(token_mask):
    # LE=64 packs two heads per 128-row contraction tile (legal {0,64}
    # PSUM offsets); requires every batch row to have <= 64 active tokens.
    n_max = int(np.asarray(token_mask).sum(axis=1).max())
    assert n_max <= 64, f"head-packed kernel needs <=64 active tokens, got {n_max}"
    return 64


def prep_core_inputs(visual_feat, text_feat, token_mask, wq, wk, wv, wo,
                     ln_gamma, ln_beta, l_eff=L):
    """Host-side prep: shard over batch, fold gamma, pack X^T fp8, cast bf16."""
    vf = np.ascontiguousarray(visual_feat.reshape(B, -1, D))
    wk2 = (ln_gamma[:, None] * wk).astype(np.float32)
    wv2 = (ln_gamma[:, None] * wv).astype(np.float32)
    wqt_b = np.ascontiguousarray(
        np.transpose(wq.reshape(NCH, P, H, P), (3, 2, 0, 1))
    ).astype(np.float16)
    wk_b = wk2.astype(np.float16)
    wv_b = wv2.astype(np.float16)
    wo_b = wo.astype(np.float16)

    in_maps = []
    for b in range(B):
        xt = pack_xt_fp8(vf[b])
        m = np.asarray(token_mask[b], bool)
        n = int(m.sum())
        # pack unmasked tokens only; zero pad rows give tn=0 -> K=V=0 -> E=1,
        # corrected by negcnt (attention is permutation-invariant over keys)
        text = np.zeros((P, D), np.float32)
        text[:n] = text_feat[b][m]
        negcnt = np.full((P, 1), -(l_eff - n), np.float32)
        in_maps.append({
            "xt": xt, "text": text, "negcnt": negcnt,
            "wqt": wqt_b, "wk": wk_b, "wv": wv_b, "wo": wo_b,
        })
    # LN beta correction: beta affects scores only via a softmax-invariant
    # per-row constant, and the output via a constant row added everywhere.
    out_corr = (ln_beta.astype(np.float64) @ wv2.astype(np.float64)
                @ wo.astype(np.float64)).astype(np.float32)
    return in_maps, out_corr


def kernel(visual_feat, text_feat, token_mask, Wq, Wk, Wv, Wo, ln_gamma, ln_beta):
    global LAST_RESULTS
    visual_feat = np.asarray(visual_feat, np.float32)
    text_feat = np.asarray(text_feat, np.float32)
    token_mask = np.asarray(token_mask)

    l_eff = pick_l_eff(token_mask)
    in_maps, out_corr = prep_core_inputs(
        visual_feat, text_feat, token_mask,
        np.asarray(Wq, np.float32), np.asarray(Wk, np.float32),
        np.asarray(Wv, np.float32), np.asarray(Wo, np.float32),
        np.asarray(ln_gamma, np.float32), np.asarray(ln_beta, np.float32),
        l_eff=l_eff,
    )
    nc = _get_program(NQ, l_eff)
    res = run_bass_kernel_spmd(nc, in_maps, core_ids=list(range(N_CORES)))
    LAST_RESULTS = res
    out = np.stack([res.results[b]["out"].astype(np.float32) for b in range(B)], axis=0)
    if np.any(out_corr):
        out = out + out_corr[None, None, :]
    return out.reshape(B, T, S_, D)


# revision 25
# speedup vs baseline: 1.1469x; 1.1469x over previous
"""Fused cross-attention kernel for Trainium2, data-parallel over batch on 8 cores.

Per core (one batch element), using the low-rank structure of cross-attention
(L=77 << D=512):
  W2_h = Wq_h @ K_h^T  [512, 77] -> scores_h = X @ W2_h     (fp8 DoubleRow)
  W3_h = V_h @ Wo_h    [77, 512] -> out = sum_h attn_h^T.T @ W3_h  (fp16)
Main loop processes PAIRS of 128-row q-tiles through a 7-stage software
pipeline so the PE never waits on the softmax chain, and per-instruction
fixed costs (scalar ACTIVATE is (N+352)/1.2 ns) amortize over 2 tiles:
  S0 scores (PE)  S1 exp x2 (Scalar)  S2 reduce (Vector) + negcnt (GpSimd)
  S3 recip (Vector) + normalize (GpSimd)  S4 transposes (PE) + copy (Scalar)
  S5 out matmul (PE)  S6 out copies (Vector) + DMA
"""

import sys

sys.path.insert(0, "/opt/trn_rl_repo")

import numpy as np
import ml_dtypes

import concourse.bass as bass
import concourse.mybir as mybir
import concourse.tile as tile
from concourse import bacc
from concourse.bass_utils import run_bass_kernel_spmd
from concourse.masks import make_identity

N_CORES = 8
B, T, S_, D, L, H = 8, 64, 196, 512, 77, 4
DH = D // H  # 128
NQ = T * S_  # 12544
LN_EPS = 1e-6
SCALE = float(DH) ** -0.5
P = 128
NCH = D // P  # 4 chunks of the feature dim
W2S = 8.0  # fp8 pre-scale for W2 (folded back out via exp's scale)
GT = 4  # q-tiles per DMA group (2KB per partition line)

F32 = mybir.dt.float32
BF16 = mybir.dt.bfloat16
FP16 = mybir.dt.float16
FP8 = mybir.dt.float8e4
DR = mybir.MatmulPerfMode.DoubleRow

LAST_RESULTS = None  # BassKernelResults of the most recent run (for test harness)
_PROGRAM_CACHE = {}


def build_program(nq=NQ, l_eff=L):
    nc = bacc.Bacc("TRN2", target_bir_lowering=False, debug=False, num_devices=N_CORES)

    LE = l_eff
    xt = nc.dram_tensor("xt", [nq * D], FP8, kind="ExternalInput").ap()
    text = nc.dram_tensor("text", [P, D], F32, kind="ExternalInput").ap()
    negcnt = nc.dram_tensor("negcnt", [P, 1], F32, kind="ExternalInput").ap()
    wqt = nc.dram_tensor("wqt", [P, H, NCH, P], FP16, kind="ExternalInput").ap()
    wk = nc.dram_tensor("wk", [D, D], FP16, kind="ExternalInput").ap()
    wv = nc.dram_tensor("wv", [D, D], FP16, kind="ExternalInput").ap()
    wo = nc.dram_tensor("wo", [D, D], FP16, kind="ExternalInput").ap()
    out = nc.dram_tensor("out", [nq, D], FP16, kind="ExternalOutput").ap()

    ntiles = nq // P  # 98
    npairs = ntiles // 2  # 49
    ngroups = (ntiles + GT - 1) // GT

    def group_span(g):
        t0 = g * GT
        return t0, min(GT, ntiles - t0)

    with tile.TileContext(nc) as tc:
        with (
            tc.tile_pool(name="const", bufs=1) as const,
            tc.tile_pool(name="xtp", bufs=3) as xtp,
            tc.tile_pool(name="ep", bufs=4) as ep,
            tc.tile_pool(name="atp", bufs=3) as atp,
            tc.tile_pool(name="attp", bufs=3) as attp,
            tc.tile_pool(name="smalls", bufs=24) as smalls,
            tc.tile_pool(name="outp", bufs=3) as outp,
            tc.tile_pool(name="ps_sc", bufs=4, space="PSUM") as ps_sc,
            tc.tile_pool(name="ps_at", bufs=2, space="PSUM") as ps_at,
            tc.tile_pool(name="ps_out", bufs=2, space="PSUM") as ps_out,
        ):
            # ---- constants / prolog ----
            ident16 = const.tile([P, P], FP16)
            make_identity(nc, ident16)
            wk_sb = const.tile([P, NCH, D], FP16, tag="wk")
            wv_sb = const.tile([P, NCH, D], FP16, tag="wv")
            wo_sb = const.tile([P, NCH, D], FP16, tag="wo")
            text_sb = const.tile([P, D], F32, tag="text")
            nc.sync.dma_start(out=text_sb[:], in_=text)
            nc.sync.dma_start(
                out=wk_sb[:], in_=wk.rearrange("(c p) n -> p c n", p=P)
            )
            wqT_sb = const.tile([P, H, NCH, P], FP16, tag="wqT")
            nc.sync.dma_start(out=wqT_sb[:], in_=wqt)

            # X prefetch of the first two groups ahead of the wv/wo DMAs
            xt_tiles = {}

            def dma_group(g):
                t0, gt = group_span(g)
                xt_sb = xtp.tile([P, NCH, GT * P], FP8, tag="xt")
                qg = gt * P
                in_ap = bass.AP(
                    tensor=xt.tensor,
                    offset=t0 * P * D,
                    ap=[[NCH * qg, P], [qg, NCH], [1, qg]],
                )
                nc.sync.dma_start(out=xt_sb[:, :, :qg], in_=in_ap)
                xt_tiles[g] = xt_sb

            dma_group(0)
            dma_group(1)
            for w_sb, w_dram in ((wv_sb, wv), (wo_sb, wo)):
                nc.sync.dma_start(
                    out=w_sb[:], in_=w_dram.rearrange("(c p) n -> p c n", p=P)
                )

            negcnt_sb = const.tile([P, 1], F32, tag="negcnt")
            nc.sync.dma_start(out=negcnt_sb[:], in_=negcnt)

            # ---- LayerNorm of text (77 valid rows; pad rows are zeros) ----
            stats = smalls.tile([P, 6], F32, tag="bnstats")
            nc.vector.bn_stats(out=stats[:], in_=text_sb[:])
            mv = smalls.tile([P, 2], F32, tag="bnaggr")
            nc.vector.bn_aggr(out=mv[:], in_=stats[:])
            eps_sb = smalls.tile([P, 1], F32, tag="eps")
            nc.vector.memset(eps_sb[:], LN_EPS)
            std = smalls.tile([P, 1], F32, tag="std")
            nc.scalar.activation(
                std[:], mv[:, 1:2], mybir.ActivationFunctionType.Sqrt, bias=eps_sb[:]
            )
            rstd = smalls.tile([P, 1], F32, tag="rstd")
            nc.vector.reciprocal(rstd[:], std[:])
            tn_sb = const.tile([P, D], FP16, tag="tn")
            nc.vector.tensor_scalar(
                out=tn_sb[:],
                in0=text_sb[:],
                scalar1=mv[:, 0:1],
                scalar2=rstd[:],
                op0=mybir.AluOpType.subtract,
                op1=mybir.AluOpType.mult,
            )

            # ---- tn^T (D on partitions) ----
            tnT_sb = const.tile([P, NCH, P], FP16, tag="tnT")
            for c in range(NCH):
                ps_t = ps_at.tile([P, 2 * H * P], FP16, tag="ps_tr")
                nc.tensor.transpose(ps_t[:, :P], tn_sb[:, c * P : (c + 1) * P], ident16[:])
                nc.scalar.copy(tnT_sb[:, c, :], ps_t[:, :P])

            # ---- K^T (masked cols) and V^T, feature dim on partitions ----
            kt_sb = const.tile([P, H, LE], FP16, tag="kt")
            vt_sb = const.tile([P, H, LE], FP16, tag="vt")
            for w_sb_, dst in ((wk_sb, kt_sb), (wv_sb, vt_sb)):
                for dch in range(NCH):
                    ps_k = ps_sc.tile([P, H * L], F32, tag="ps_s")
                    for kc in range(NCH):
                        nc.tensor.matmul(
                            ps_k[:, :LE],
                            w_sb_[:, kc, dch * P : (dch + 1) * P],
                            tnT_sb[:, kc, :LE],
                            start=(kc == 0),
                            stop=(kc == NCH - 1),
                        )
                    nc.vector.tensor_copy(dst[:, dch, :], ps_k[:, :LE])

            # ---- W2_h = Wq_h @ K_h^T -> fp8 * W2S, layout [p, c, h*L] ----
            w2_sb = const.tile([P, NCH, H, LE], FP8, tag="w2")
            for h in range(H):
                for dch in range(NCH):
                    ps_w = ps_sc.tile([P, H * L], F32, tag="ps_s")
                    nc.tensor.matmul(
                        ps_w[:, :LE], wqT_sb[:, h, dch, :], kt_sb[:, h, :],
                        start=True, stop=True,
                    )
                    nc.scalar.activation(
                        w2_sb[:, dch, h, :], ps_w[:, :LE],
                        mybir.ActivationFunctionType.Copy, scale=W2S,
                    )

            # ---- W3: heads packed in pairs, head 2hp+s at partition offset
            # s*LE of tile hp (LE=64 keeps offsets in the legal {0,64} set),
            # so the out matmul contracts two heads in one 128-row matmul ----
            w3_sb = const.tile([P, 2, D], FP16, tag="w3")

            def build_w3():
                for hp in range(2):
                    ps_w = ps_out.tile([P, D], F32, tag="ps_o")
                    for s in range(2):
                        nc.tensor.matmul(
                            ps_w[s * LE : (s + 1) * LE],
                            vt_sb[:, 2 * hp + s, :],
                            wo_sb[:, 2 * hp + s, :],
                            start=True, stop=True,
                        )
                    nc.scalar.copy(w3_sb[:, hp, :], ps_w[:])

            # ---- main loop: pairs of q-tiles through a 7-stage pipeline ----
            E_t, attn_t, attnT_t, sum_t, adj_t, recip_t = {}, {}, {}, {}, {}, {}
            ps_s_t, ps_o_t = {}, {}

            for i in range(npairs + 7):
                if i == 4:
                    build_w3()
                # S0: scores on PE (fp8 DoubleRow), 2 tiles
                if i < npairs:
                    if i % 2 == 0:
                        gnext = i // 2 + 2
                        if gnext < ngroups:
                            dma_group(gnext)
                    for t in (2 * i, 2 * i + 1):
                        g = t // GT
                        tq = slice((t - g * GT) * P, (t - g * GT + 1) * P)
                        ps_s = ps_sc.tile([P, H * L], F32, tag="ps_s")
                        for j in range(2):
                            nc.tensor.matmul(
                                ps_s[:, : H * LE],
                                xt_tiles[g][:, 2 * j : 2 * j + 2, tq],
                                w2_sb[:, 2 * j : 2 * j + 2, :, :].rearrange(
                                    "p c h n -> p c (h n)"
                                ),
                                start=(j == 0),
                                stop=(j == 1),
                                perf_mode=DR,
                            )
                        ps_s_t[t] = ps_s

                # S1: exp x2 on Scalar -> E fp16 (both tiles of pair in one tile)
                p = i - 1
                if 0 <= p < npairs:
                    E_sb = ep.tile([P, 2, H, LE], FP16, tag="exp")
                    for j in range(2):
                        t = 2 * p + j
                        nc.scalar.activation(
                            E_sb[:, j],
                            ps_s_t[t][:, : H * LE].rearrange("p (c n) -> p c n", c=H),
                            mybir.ActivationFunctionType.Exp, scale=SCALE / W2S,
                        )
                        del ps_s_t[t]
                    E_t[p] = E_sb

                # S2: reduce on Vector; +negcnt on GpSimd
                p = i - 2
                if 0 <= p < npairs:
                    sumexp = smalls.tile([P, 2 * H], F32, tag="sumexp")
                    nc.vector.reduce_sum(
                        out=sumexp[:], in_=E_t[p][:], axis=mybir.AxisListType.X
                    )
                    sumadj = smalls.tile([P, 2 * H], F32, tag="sumadj")
                    nc.gpsimd.tensor_scalar_add(sumadj[:], sumexp[:], negcnt_sb[:])
                    adj_t[p] = sumadj

                # S3: recip on Vector; normalize on GpSimd -> attn fp16
                p = i - 3
                if 0 <= p < npairs:
                    recip = smalls.tile([P, 2 * H], F32, tag="recip")
                    nc.vector.reciprocal_approx_fast(recip[:], adj_t[p][:])
                    del adj_t[p]
                    attn_sb = atp.tile([P, 2, H, LE], FP16, tag="attn")
                    nc.gpsimd.tensor_mul(
                        attn_sb[:], E_t[p][:],
                        recip[:].rearrange("p (j c) -> p j c", j=2).to_broadcast(
                            [P, 2, H, LE]
                        ),
                    )
                    attn_t[p] = attn_sb
                    del E_t[p]

                # S4: transposes on PE (two heads per transpose: input
                # [128, 2*LE=128], output rows = packed head pair); copy on Scalar
                p = i - 4
                if 0 <= p < npairs:
                    ps_a = ps_at.tile([P, 2 * 2 * P], FP16, tag="ps_tr")
                    for j in range(2):
                        for hp in range(2):
                            sec = j * 2 + hp
                            nc.tensor.transpose(
                                ps_a[:, sec * P : (sec + 1) * P],
                                attn_t[p][:, j, 2 * hp : 2 * hp + 2, :].rearrange(
                                    "p a b -> p (a b)"
                                ),
                                ident16[:],
                            )
                    attnT_sb = attp.tile([P, 2, 2, P], FP16, tag="attnT")
                    nc.vector.tensor_copy(
                        attnT_sb[:].bitcast(mybir.dt.uint32),
                        ps_a[:].bitcast(mybir.dt.uint32),
                    )
                    attnT_t[p] = attnT_sb
                    del attn_t[p]

                # S5: out matmuls on PE (2 tiles, separate PSUM banks)
                p = i - 5
                if 0 <= p < npairs:
                    for j in range(2):
                        ps_o = ps_out.tile([P, D], F32, tag="ps_o")
                        for hp in range(2):
                            nc.tensor.matmul(
                                ps_o[:],
                                attnT_t[p][:, j, hp, :],
                                w3_sb[:, hp, :],
                                start=(hp == 0),
                                stop=(hp == 1),
                            )
                        ps_o_t[2 * p + j] = ps_o
                    del attnT_t[p]

                # S6: out copies on Vector, one pair-DMA out
                p = i - 6
                if 0 <= p < npairs:
                    out_sb = outp.tile([P, 2, D], FP16, tag="out")
                    nc.scalar.copy(out_sb[:, 0], ps_o_t[2 * p][:])
                    del ps_o_t[2 * p]
                    nc.vector.tensor_copy(out_sb[:, 1], ps_o_t[2 * p + 1][:])
                    del ps_o_t[2 * p + 1]
                    out_ap = bass.AP(
                        tensor=out.tensor,
                        offset=2 * p * P * D,
                        ap=[[D, P], [P * D, 2], [1, D]],
                    )
                    nc.sync.dma_start(out=out_ap, in_=out_sb[:])
                    g_done = (2 * p) // GT
                    if 2 * p + 1 == min(ntiles, (g_done + 1) * GT) - 1:
                        del xt_tiles[g_done]

    nc.compile()
    return nc


def _get_program(nq=NQ, l_eff=L):
    key = (nq, l_eff)
    if key not in _PROGRAM_CACHE:
        _PROGRAM_CACHE[key] = build_program(nq, l_eff)
    return _PROGRAM_CACHE[key]


def pack_xt_fp8(x):
    """[NQ, D] f32 -> flat fp8 in per-group [p, c, q] layout (2KB DMA lines)."""
    xtT = np.ascontiguousarray(x.T).astype(ml_dtypes.float8_e4m3fn)  # [D, NQ]
    ntiles = x.shape[0] // P
    flat = np.empty(x.size, ml_dtypes.float8_e4m3fn)
    t0 = 0
    while t0 < ntiles:
        gt = min(GT, ntiles - t0)
        q0, q1 = t0 * P, (t0 + gt) * P
        blk = xtT[:, q0:q1].reshape(NCH, P, gt * P).transpose(1, 0, 2)
        flat[q0 * D : q1 * D] = blk.ravel()
        t0 += gt
    return flat


def pick_l_eff(token_mask):
    # LE=64 packs two heads per 128-row contraction tile (legal {0,64}
    # PSUM offsets); requires every batch row to have <= 64 active tokens.
    n_max = int(np.asarray(token_mask).sum(axis=1).max())
    assert n_max <= 64, f"head-packed kernel needs <=64 active tokens, got {n_max}"
    return 64


def prep_core_inputs(visual_feat, text_feat, token_mask, wq, wk, wv, wo,
                     ln_gamma, ln_beta, l_eff=L):
    """Host-side prep: shard over batch, fold gamma, pack X^T fp8, cast bf16."""
    vf = np.ascontiguousarray(visual_feat.reshape(B, -1, D))
    wk2 = (ln_gamma[:, None] * wk).astype(np.float32)
    wv2 = (ln_gamma[:, None] * wv).astype(np.float32)
    wqt_b = np.ascontiguousarray(
        np.transpose(wq.reshape(NCH, P, H, P), (3, 2, 0, 1))
    ).astype(np.float16)
    wk_b = wk2.astype(np.float16)
    wv_b = wv2.astype(np.float16)
    wo_b = wo.astype(np.float16)

    in_maps = []
    for b in range(B):
        xt = pack_xt_fp8(vf[b])
        m = np.asarray(token_mask[b], bool)
        n = int(m.sum())
        # pack unmasked tokens only; zero pad rows give tn=0 -> K=V=0 -> E=1,
        # corrected by negcnt (attention is permutation-invariant over keys)
        text = np.zeros((P, D), np.float32)
        text[:n] = text_feat[b][m]
        negcnt = np.full((P, 1), -(l_eff - n), np.float32)
        in_maps.append({
            "xt": xt, "text": text, "negcnt": negcnt,
            "wqt": wqt_b, "wk": wk_b, "wv": wv_b, "wo": wo_b,
        })
    # LN beta correction: beta affects scores only via a softmax-invariant
    # per-row constant, and the output via a constant row added everywhere.
    out_corr = (ln_beta.astype(np.float64) @ wv2.astype(np.float64)
                @ wo.astype(np.float64)).astype(np.float32)
    return in_maps, out_corr


def kernel(visual_feat, text_feat, token_mask, Wq, Wk, Wv, Wo, ln_gamma, ln_beta):
    global LAST_RESULTS
    visual_feat = np.asarray(visual_feat, np.float32)
    text_feat = np.asarray(text_feat, np.float32)
    token_mask = np.asarray(token_mask)

    l_eff = pick_l_eff(token_mask)
    in_maps, out_corr = prep_core_inputs(
        visual_feat, text_feat, token_mask,
        np.asarray(Wq, np.float32), np.asarray(Wk, np.float32),
        np.asarray(Wv, np.float32), np.asarray(Wo, np.float32),
        np.asarray(ln_gamma, np.float32), np.asarray(ln_beta, np.float32),
        l_eff=l_eff,
    )
    nc = _get_program(NQ, l_eff)
    res = run_bass_kernel_spmd(nc, in_maps, core_ids=list(range(N_CORES)))
    LAST_RESULTS = res
    out = np.stack([res.results[b]["out"].astype(np.float32) for b in range(B)], axis=0)
    if np.any(out_corr):
        out = out + out_corr[None, None, :]
    return out.reshape(B, T, S_, D)
